# revision 35
# baseline (speedup 1.0000x reference)
"""Multi-head attention (B=4, S=2048, D=1024, H=16) on 8 Trainium2 NeuronCores.

Sharding: core c -> batch c//2, head-group c%2 (8 heads = 512 dims each).
Each core computes qkv projection, softmax attention and its partial
out-projection (Megatron row-split of w_out); the host sums core pairs.

All projection/attention operands are bf16 with fp32 PSUM accumulation
(x, qkv weights, qT/kT, v, exp(scores)); only the out-projection stays
float32r.  bf16 halves DMA bytes (the DMA engines are packet-rate bound:
~288ns per <=2KB line, so all big transfers use >=2KB lines) and makes
every hot-loop LDWEIGHTS a standalone fast-load that hides behind
matmuls.  x lives resident in SBUF (4MB bf16) so phase-B fillers do no
DMA.  Softmax needs no max-subtraction (scores ~ N(0,1)); denominators
come free from an augmented ones-column in V; the reciprocal runs on a
[128,4] DRAM-reshaped layout (DVE reciprocal cost scales with free
size), and the normalization multiply runs on the idle GPSIMD.

Schedule (profiled ~366us; PE-timeline-bound: ~786K PE cycles at
2.4GHz under a ~95% power-throttle duty ceiling, ACT ~78% busy):
  - pass 1 (~51us): one sweep over resident x computes v (all heads) +
    q/k (pair 0); loads spread over all three DMA-capable queues
    (sync/scalar/gpsimd run concurrently, ~400GB/s aggregate); chunk 0
    interleaves qk+v per d in DMA-arrival order, later chunks run all
    qk matmuls first so the previous chunk's psv drains overlap them.
  - phase B (~272us): per head-pair, scores (row-tiled 2-head pack) ->
    one exp per [128,1024] psum tile -> pv accumulation lagging 2
    iterations (pv LDWEIGHTS carries no pending wait); filler matmuls
    project the next pair's q/k from resident x in the PE slack of
    pairs 0-2 (PE-bound at 1066ns/iter vs 1038ns exp).  Attention
    output lands in per-(pair, 512-col q-chunk) oT tiles so the
    out-projection can consume chunks as they normalize.
  - out-projection (32 units of (dout-block j, q-chunk tc): 4 po
    matmuls + bias + [128,512] bf16 store): ~20 matmuls absorbed into
    pair 3's ACT-bound slack (no qk fillers there; 6 insert slots/qu,
    psum from the idle filler tags, DVE bias, gpsimd-queue DMA); the
    remaining ~110 run densely after the pr loop on freed attention
    psum tags, bias alternating ACT/DVE, stores alternating sync/
    scalar queues, tc ascending so the last-normalized chunk is last.
"""

import numpy as np

B, S, D, H = 4, 2048, 1024, 16
HD = D // H          # 64
HG = H // 2          # 8 heads per core
DG = HG * HD         # 512 local head-cat dims
SCALE = HD ** -0.5   # folded into wq host-side
NCORES = 8

_CACHE = {}


# --------------------------------------------------------------------------
# wait splitting: this toolchain's walrus rejects >1 sync wait per instruction
# on some paths; move excess semaphore waits onto same-engine NoOps.
# --------------------------------------------------------------------------
def _split_excess_waits(nc, max_waits=1):
    import bass_rust
    import concourse.mybir as mybir

    ctr = [0]
    for fn in nc.m.functions:
        for bb in fn.blocks:
            insts = list(bb.instructions)
            out = []
            changed = False
            for inst in insts:
                si = inst.sync_info
                waits = list(si.on_wait) if si is not None and si.on_wait else []
                sem_waits = [w for w in waits if w.sync_type == "semaphore"]
                other = [w for w in waits if w.sync_type != "semaphore"]
                budget = max_waits - len(other)
                if len(sem_waits) > budget and budget >= 1:
                    head, keep = sem_waits[:-budget], sem_waits[-budget:]
                    chunks = [
                        head[i : i + max_waits]
                        for i in range(0, len(head), max_waits)
                    ]
                    for ch in chunks:
                        nop = mybir.InstNoOp(
                            name=f"wsplit-{ctr[0]}",
                            opcode="NoOp",
                            engine=inst.engine,
                            ins=[],
                            outs=[],
                        )
                        nop.sync_info = bass_rust.SyncInfo(on_wait=ch, on_update=[])
                        ctr[0] += 1
                        out.append(nop)
                    inst.sync_info = bass_rust.SyncInfo(
                        on_wait=other + keep,
                        on_update=list(si.on_update) if si.on_update else [],
                    )
                    changed = True
                out.append(inst)
            if changed:
                bb.instructions = out


# --------------------------------------------------------------------------
# device program (identical on all 8 cores)
# --------------------------------------------------------------------------
def _build(split_waits=True):
    import concourse.bass as bass
    import concourse.tile as tile
    import concourse.mybir as mybir

    F32 = mybir.dt.float32
    F32R = mybir.dt.float32r
    BF16 = mybir.dt.bfloat16
    EXP = mybir.ActivationFunctionType.Exp
    IDENT = mybir.ActivationFunctionType.Identity
    ts = bass.ts

    nc = bass.Bass()

    xb = nc.dram_tensor("xb", [D, S], BF16, kind="ExternalInput")
    # wqk DRAM layout: cols 0:512 = wq (4 pairs x 128), 512:1024 = wk;
    # consumed only by phase-B fillers, so it loads off the critical path.
    wqk = nc.dram_tensor("wqk", [D, 2 * DG], BF16, kind="ExternalInput")
    # pass-1-critical weights, host-repacked d-major so chunk 0's blocking
    # DMA is 0.5MB+1MB of 2KB lines instead of 3MB:
    #   wq0p[p, d*256+c]  = pair-0 q (c<128) / k (c-128) col c, row d*128+p
    #   wvpk[p, d*512+c]  = wv col c, row d*128+p
    wq0p = nc.dram_tensor("wq0p", [128, (D // 128) * 256], BF16, kind="ExternalInput")
    wvpk = nc.dram_tensor("wvpk", [128, (D // 128) * DG], BF16, kind="ExternalInput")
    bqk = nc.dram_tensor("bqk", [128, 8], F32, kind="ExternalInput")
    bv = nc.dram_tensor("bv", [128, DG], F32, kind="ExternalInput")
    wo = nc.dram_tensor("wo", [DG, D], F32R, kind="ExternalInput")
    bo = nc.dram_tensor("bo", [128, D // 128], F32, kind="ExternalInput")
    outp = nc.dram_tensor("outp", [D, S], BF16, kind="ExternalOutput")

    NSQT = S // 128          # 16 sq/sk tiles of 128
    NDT = D // 128           # 8 contraction tiles
    NPAIR = HG // 2          # 4 head pairs
    VW = HD + 1              # 65: v columns + ones column per head
    SQQ = 512                # sq quarter per pv accumulation

    with tile.TileContext(nc) as tc:
        with (
            tc.tile_pool(name="bias", bufs=1) as bias_pool,
            tc.tile_pool(name="vaug", bufs=1) as v_pool,
            tc.tile_pool(name="oT", bufs=1) as oT_pool,
            tc.tile_pool(name="qkp", bufs=1) as qkp,      # rotating qT/kT slots
            tc.tile_pool(name="wqkp", bufs=1) as wqkp,
            tc.tile_pool(name="xs2", bufs=1) as xs2,      # resident bf16 x [D,S]
            tc.tile_pool(name="psqk", bufs=1, space="PSUM") as ps_qk,  # pa,pb
        ):
            bqk_t = bias_pool.tile([128, 8], F32)
            bv_t = bias_pool.tile([128, DG], F32)
            ones8_f = bias_pool.tile([128, 8], F32)
            nc.vector.memset(ones8_f[:], 1.0)
            ones8 = bias_pool.tile([128, 8], BF16)
            nc.vector.tensor_copy(ones8[:], ones8_f[:])
            actwarm = bias_pool.tile([1, 1], F32)

            # wqk_t[d]: cols 0:512 = wq (SCALE folded), 512:1024 = wk
            # (whole-tile loads: 2KB DMA lines; splitting by pair halves the
            # critical bytes but quadruples packets -> slower, measured)
            wqk_t = [
                wqkp.tile([128, 2 * DG], BF16, name=f"wqk{d}", tag=f"wqk{d}")
                for d in range(NDT)
            ]

            v_t = [v_pool.tile([128, HG * VW], BF16, name=f"v{s}", tag=f"v{s}") for s in range(NSQT)]
            # attention output, one tile per (pair, 512-col q-chunk) so the
            # out-projection can start on a chunk as soon as its normalize
            # lands (tile-granular deps)
            oTq = [
                [
                    oT_pool.tile([128, SQQ], F32R, name=f"oT{p}_{qc}", tag=f"oT{p}_{qc}")
                    for qc in range(S // SQQ)
                ]
                for p in range(NPAIR)
            ]
            qT_t = [qkp.tile([128, S], BF16, name=f"qT{i}", tag=f"qT{i}") for i in range(2)]
            kT_t = [qkp.tile([128, S], BF16, name=f"kT{i}", tag=f"kT{i}") for i in range(2)]

            # ------- pass 1: single sweep over x -> v(all) + qk(pair0) -------
            # x arrives as [128,1024] bf16 tiles (2KB DMA lines), each
            # resident across the two 512-col chunks that consume it; per
            # chunk+d we issue 6 back-to-back matmuls.  PSUM: gq/gk double-
            # buffered across chunks (pa,pb | pg,ph) + 4 v banks = 8.
            with (
                tc.tile_pool(name="wvp", bufs=1) as wvp,
                tc.tile_pool(name="psv", bufs=1, space="PSUM") as ps_v,
                tc.tile_pool(name="psg", bufs=1, space="PSUM") as ps_g,
            ):
                # packed pass-1 weights, 2 tiles each (d 0-3 / d 4-7) so the
                # second half's arrival doesn't gate the first half's use
                wq0_t = [
                    wvp.tile([128, 4 * 256], BF16, name=f"wq0_{j}", tag=f"wq0_{j}")
                    for j in range(2)
                ]
                wvk_t = [
                    wvp.tile([128, 4 * DG], BF16, name=f"wvk{j}", tag=f"wvk{j}")
                    for j in range(2)
                ]
                # x stays resident in SBUF for the whole kernel (the phase-B
                # fillers then need no DMA at all); weight and x loads are
                # emitted d-interleaved in demand order across both hw queues
                xr_t = [
                    [
                        xs2.tile([128, 1024], BF16, name=f"x2_{cc}_{d}", tag=f"x{cc}_{d}")
                        for d in range(NDT)
                    ]
                    for cc in range(S // 1024)
                ]
                # three concurrent DMA queues (sync/scalar/gpsimd-SWDGE
                # aggregate ~360GB/s); cc0-critical tensors first, cc1's x
                # trails so it never competes with the cc0 window
                # chunk-0-critical stream: packed qk-pair0 + packed wv +
                # x chunk 0, ~3.5MB of 2KB lines vs ~10us of chunk-0 compute
                nc.sync.dma_start(wq0_t[0][:], wq0p[:, 0:1024])
                nc.gpsimd.dma_start(wvk_t[0][:], wvpk[:, 0 : 4 * DG])
                for d in range(NDT):
                    eng = nc.sync if d % 2 == 0 else nc.scalar
                    eng.dma_start(xr_t[0][d][:], xb[ts(d, 128), ts(0, 1024)])
                    if d == 1:
                        nc.sync.dma_start(wq0_t[1][:], wq0p[:, 1024:2048])
                        nc.gpsimd.dma_start(wvk_t[1][:], wvpk[:, 4 * DG : 8 * DG])
                for d in range(NDT):
                    eng = nc.sync if d % 2 == 0 else nc.scalar
                    eng.dma_start(xr_t[1][d][:], xb[ts(d, 128), ts(1, 1024)])
                # non-critical loads behind the pass-1 stream: biases (first
                # needed at the ci=0 drain), the one-time exp table preload
                # (~2.7us, must land before phase B), and the filler-only
                # full wqk tiles (first needed at phase B, ~60us in)
                nc.gpsimd.dma_start(bqk_t[:], bqk[:, :])
                nc.gpsimd.dma_start(bv_t[:], bv[:, :])
                nc.scalar.activation(actwarm[:], ones8_f[0:1, 0:1], EXP)
                for d in range(NDT):
                    eng = nc.sync if d % 2 == 0 else nc.scalar
                    eng.dma_start(wqk_t[d][:], wqk[ts(d, 128), :])

                for cc in range(S // 1024):
                    x2_t = xr_t[cc]
                    for sub in range(2):
                        ci = 2 * cc + sub
                        if ci % 2 == 0:
                            gq = ps_qk.tile([128, 512], F32, name="gq", tag="pa")
                            gk = ps_qk.tile([128, 512], F32, name="gk", tag="pb")
                        else:
                            gq = ps_g.tile([128, 512], F32, name="gq", tag="pg")
                            gk = ps_g.tile([128, 512], F32, name="gk", tag="ph")
                        psv = [
                            ps_v.tile([128, DG], F32, name="psv", tag=t)
                            for t in ("pc", "pd", "pe", "pf")
                        ]
                        def _wq0(d):
                            return wq0_t[d // 4][:, (d % 4) * 256 : (d % 4) * 256 + 128]

                        def _wk0(d):
                            return wq0_t[d // 4][:, (d % 4) * 256 + 128 : (d % 4) * 256 + 256]

                        def _wv(d):
                            return wvk_t[d // 4][:, (d % 4) * DG : (d % 4 + 1) * DG]

                        if ci == 0:
                            # chunk 0 is DMA-paced: consume tensors in
                            # arrival order, qk+v interleaved per d
                            for d in range(NDT):
                                xch = x2_t[d][:, ts(sub, 512)]
                                nc.tensor.matmul(
                                    gq[:], _wq0(d), xch,
                                    start=(d == 0), stop=(d == NDT - 1),
                                )
                                nc.tensor.matmul(
                                    gk[:], _wk0(d), xch,
                                    start=(d == 0), stop=(d == NDT - 1),
                                )
                                for si in range(4):
                                    nc.tensor.matmul(
                                        psv[si][:], xch[:, ts(si, 128)], _wv(d),
                                        start=(d == 0), stop=(d == NDT - 1),
                                    )
                        else:
                            # later chunks: qk first so the previous chunk's
                            # psv drains (4 x 683ns on DVE) overlap ~3.4us of
                            # qk matmuls instead of stalling the psv writes
                            for d in range(NDT):
                                xch = x2_t[d][:, ts(sub, 512)]
                                nc.tensor.matmul(
                                    gq[:], _wq0(d), xch,
                                    start=(d == 0), stop=(d == NDT - 1),
                                )
                                nc.tensor.matmul(
                                    gk[:], _wk0(d), xch,
                                    start=(d == 0), stop=(d == NDT - 1),
                                )
                            for d in range(NDT):
                                xch = x2_t[d][:, ts(sub, 512)]
                                for si in range(4):
                                    nc.tensor.matmul(
                                        psv[si][:], xch[:, ts(si, 128)], _wv(d),
                                        start=(d == 0), stop=(d == NDT - 1),
                                    )
                        # drains: v-aug first (next chunk's psv matmuls wait
                        # on these); qk bias adds last (banks double-buffered)
                        for si in range(4):
                            s = 4 * ci + si
                            vap = v_t[s][:].rearrange("p (h e) -> p h e", e=VW)
                            nc.vector.tensor_add(
                                vap[:, :, 0:HD],
                                psv[si][:].rearrange("p (h e) -> p h e", e=HD),
                                bv_t[:].rearrange("p (h e) -> p h e", e=HD),
                            )
                            nc.gpsimd.tensor_copy(
                                vap[:, :, HD : HD + 1], ones8[:, :, None]
                            )
                        nc.vector.tensor_scalar_add(
                            qT_t[0][:, ts(ci, 512)], gq[:], bqk_t[:, 0:1]
                        )
                        nc.vector.tensor_scalar_add(
                            kT_t[0][:, ts(ci, 512)], gk[:], bqk_t[:, 4:5]
                        )

            # ---------------- phase B: attention + fillers + tail ---------
            # Two heads share one [128,1024] scores psum tile; one exp covers
            # both heads.  pv matmuls are software-pipelined one step behind.
            # The next pair's q/k projection fills the PE slack under the
            # ACT-bound exp stream; x tiles are re-read as [128,1024] bf16
            # spanning two qu's.  The out-projection tail lives inside this
            # pool scope (reusing attention PSUM tags) so no pool-exit
            # barrier precedes it.
            with (
                tc.tile_pool(name="pt", bufs=3) as ptp,
                tc.tile_pool(name="scp", bufs=2, space="PSUM") as scp,
                tc.tile_pool(name="pvp", bufs=1, space="PSUM") as pvp,
                tc.tile_pool(name="nrm", bufs=1) as nrm,
                tc.tile_pool(name="pvs", bufs=1) as pvsp,
                tc.tile_pool(name="rs", bufs=4, space="DRAM") as rsp,
                tc.tile_pool(name="w3", bufs=1) as w3,
                tc.tile_pool(name="outb", bufs=6) as outb,
            ):
                wo_t = [
                    w3.tile([128, D], F32R, name=f"wo{pp}", tag=f"wo{pp}")
                    for pp in range(NPAIR)
                ]
                bo_t = outb.tile([128, NDT], F32, name="bo_t", tag="bo_t")

                # ---- out-projection emitter ---------------------------------
                # 32 units (j, tc): po[128,512] accumulated over the 4 pairs,
                # bias-add, [128,512] bf16 store.  Units whose oT chunks are
                # ready are absorbed one matmul per s-iteration into pair 3's
                # ACT-bound slack (pair 3 has no qk fillers); the rest run
                # densely after the pr loop with ACT free for bias adds.
                t_units = [(j, tc) for tc in range(S // SQQ) for j in range(NDT)]
                t_st = {"ui": 0, "pp": 0, "nd": 0, "po": None}
                slots_p3 = [(ps_qk, "pa"), (ps_qk, "pb")]
                slots_rem = [
                    (scp, "sc"), (ps_qk, "pa"), (ps_qk, "pb"),
                    (pvp, "pv0"), (pvp, "pv1"),
                ]

                def _tail_piece(in_p3, qu=None, s=None):
                    if t_st["ui"] >= len(t_units):
                        return False
                    j, tc = t_units[t_st["ui"]]
                    pp = t_st["pp"]
                    if in_p3 and pp == 0:
                        # unit-start gate: pair-3's normalize for chunk tc
                        # lands ~5-6 iterations into qu=tc+1; don't let an
                        # in-order PE wait stall the score stream
                        if not (tc <= qu - 2 or (tc == qu - 1 and s >= 4)):
                            return False
                    if pp == 0:
                        pool, tag = (slots_p3 if in_p3 else slots_rem)[
                            t_st["nd"] % (2 if in_p3 else 5)
                        ]
                        t_st["po"] = pool.tile(
                            [128, SQQ], F32, name=f"po{j}_{tc}", tag=tag
                        )
                    nc.tensor.matmul(
                        t_st["po"][:],
                        wo_t[pp][:, ts(j, 128)],
                        oTq[pp][tc][:],
                        start=(pp == 0), stop=(pp == NPAIR - 1),
                    )
                    if pp == NPAIR - 1:
                        ob = outb.tile(
                            [128, SQQ], BF16, name=f"ob{j}_{tc}", tag="ob"
                        )
                        if in_p3 or t_st["nd"] % 2:
                            nc.vector.tensor_scalar_add(
                                ob[:], t_st["po"][:], bo_t[:, j : j + 1]
                            )
                        else:
                            nc.scalar.activation(
                                ob[:], t_st["po"][:], IDENT,
                                bias=bo_t[:, j : j + 1],
                            )
                        # absorbed units must not touch the ACT engine (exp
                        # stream) nor the sync queue (normalize chains live
                        # there); the remainder alternates sync/scalar so
                        # output transfers drain twice as fast
                        dq = (
                            nc.gpsimd
                            if in_p3
                            else (nc.sync if t_st["nd"] % 2 else nc.scalar)
                        )
                        dq.dma_start(
                            outp[ts(j, 128), tc * SQQ : (tc + 1) * SQQ], ob[:]
                        )
                        t_st["nd"] += 1
                        t_st["ui"] += 1
                        t_st["pp"] = 0
                    else:
                        t_st["pp"] = pp + 1
                    return True

                for pr in range(NPAIR):
                    h0, h1 = 2 * pr, 2 * pr + 1
                    qTc, kTc = qT_t[pr % 2], kT_t[pr % 2]
                    nxt = pr + 1 if pr + 1 < NPAIR else None
                    if pr == 2:
                        # wo on sync: its phase-B program is semaphore-paced,
                        # so the transfer lands here and not during pass 1
                        # (gpsimd would issue it immediately)
                        for pp in range(NPAIR):
                            nc.sync.dma_start(wo_t[pp][:], wo[ts(pp, 128), :])
                        nc.gpsimd.dma_start(bo_t[:], bo[:, :])
                    for qu in range(S // SQQ):
                        qs = slice(qu * SQQ, (qu + 1) * SQQ)
                        pv0 = pvp.tile([VW, SQQ], F32, name="pv0", tag="pv0")
                        pv1 = pvp.tile([VW, SQQ], F32, name="pv1", tag="pv1")
                        if nxt is not None:
                            gq = ps_qk.tile([128, 512], F32, name="gq", tag="pa")
                            gk = ps_qk.tile([128, 512], F32, name="gk", tag="pb")
                        # pv matmuls run 2 iterations behind their exp so the
                        # pv LDWEIGHTS carries no pending wait and hides
                        # behind in-flight matmuls
                        pt_q = []
                        for s in range(NSQT):
                            if nxt is not None:
                                # filler: qk projection for the next pair,
                                # 1 matmul/iter from the resident x tiles,
                                # finishing (with bias adds) at s==14
                                if s < 12:
                                    d = s // 2
                                    xch = xr_t[qu // 2][d][:, ts(qu % 2, 512)]
                                    if s % 2 == 0:
                                        nc.tensor.matmul(
                                            gq[:], wqk_t[d][:, ts(nxt, 128)], xch,
                                            start=(d == 0), stop=False,
                                        )
                                    else:
                                        nc.tensor.matmul(
                                            gk[:],
                                            wqk_t[d][:, DG + nxt * 128 : DG + (nxt + 1) * 128],
                                            xch,
                                            start=(d == 0), stop=False,
                                        )
                                elif s in (12, 13):
                                    d = s - 6
                                    xch = xr_t[qu // 2][d][:, ts(qu % 2, 512)]
                                    nc.tensor.matmul(
                                        gq[:], wqk_t[d][:, ts(nxt, 128)], xch,
                                        start=False, stop=(d == NDT - 1),
                                    )
                                    nc.tensor.matmul(
                                        gk[:],
                                        wqk_t[d][:, DG + nxt * 128 : DG + (nxt + 1) * 128],
                                        xch,
                                        start=False, stop=(d == NDT - 1),
                                    )
                                    if s == 13:
                                        nc.vector.tensor_scalar_add(
                                            qT_t[nxt % 2][:, ts(qu, 512)], gq[:],
                                            bqk_t[:, nxt : nxt + 1],
                                        )
                                        nc.vector.tensor_scalar_add(
                                            kT_t[nxt % 2][:, ts(qu, 512)], gk[:],
                                            bqk_t[:, 4 + nxt : 5 + nxt],
                                        )
                            elif s in (3, 5, 7, 9, 11, 13):
                                # pair 3 is ACT-bound (~12us of PE slack
                                # total): absorb out-projection matmuls for
                                # already-normalized q-chunks, few enough to
                                # keep the exp stream the pacer
                                _tail_piece(True, qu=qu, s=s)
                            sc = scp.tile([128, 2 * SQQ], F32, name="sc", tag="sc")
                            nc.tensor.matmul(
                                sc[:, 0:SQQ],
                                kTc[0:HD, ts(s, 128)],
                                qTc[0:HD, qs],
                                start=True, stop=True,
                            )
                            nc.tensor.matmul(
                                sc[:, SQQ : 2 * SQQ],
                                kTc[HD:128, ts(s, 128)],
                                qTc[HD:128, qs],
                                start=True, stop=True,
                            )
                            pt = ptp.tile([128, 2 * SQQ], BF16, name="pt", tag="pt")
                            nc.scalar.activation(pt[:], sc[:], EXP)
                            if len(pt_q) == 2:
                                pp_, ps_ = pt_q.pop(0)
                                nc.tensor.matmul(
                                    pv0[:], v_t[ps_][:, h0 * VW : (h0 + 1) * VW],
                                    pp_[:, 0:SQQ],
                                    start=(ps_ == 0), stop=False,
                                )
                                nc.tensor.matmul(
                                    pv1[:], v_t[ps_][:, h1 * VW : (h1 + 1) * VW],
                                    pp_[:, SQQ : 2 * SQQ],
                                    start=(ps_ == 0), stop=False,
                                )
                            pt_q.append((pt, s))
                        for pp_, ps_ in pt_q:
                            nc.tensor.matmul(
                                pv0[:], v_t[ps_][:, h0 * VW : (h0 + 1) * VW],
                                pp_[:, 0:SQQ],
                                start=False, stop=(ps_ == NSQT - 1),
                            )
                            nc.tensor.matmul(
                                pv1[:], v_t[ps_][:, h1 * VW : (h1 + 1) * VW],
                                pp_[:, SQQ : 2 * SQQ],
                                start=False, stop=(ps_ == NSQT - 1),
                            )
                        # free pv banks via psum->sbuf copy, then normalize.
                        # The denominator row [1,512] is reshaped to [128,4]
                        # through DRAM so the DVE reciprocal costs ~0.17us
                        # instead of 3.3us (DVE reciprocal is ~6.5 cyc/elem
                        # and free-size-serial); the multiply runs on GPSIMD.
                        for hh, pvx, row in ((0, pv0, 0), (1, pv1, HD)):
                            pvs = pvsp.tile([VW, SQQ], F32, name=f"pvs{hh}", tag=f"pvs{hh}")
                            nc.vector.tensor_copy(pvs[:], pvx[:])
                            rsd = rsp.tile([1, SQQ], F32, name=f"rsd{hh}", tag=f"rsd{hh}")
                            nc.sync.dma_start(rsd[:], pvs[HD : HD + 1, :])
                            dent = nrm.tile([128, 4], F32, name=f"dent{hh}", tag=f"dent{hh}")
                            nc.sync.dma_start(dent[:], rsd[:])
                            rcd = nrm.tile([128, 4], F32, name=f"rcd{hh}", tag=f"rcd{hh}")
                            nc.vector.reciprocal(rcd[:], dent[:])
                            rs2 = rsp.tile([1, SQQ], F32, name=f"rs2{hh}", tag=f"rs2{hh}")
                            nc.sync.dma_start(rs2[:], rcd[:])
                            bcs = nrm.tile([HD, SQQ], F32, name=f"bcs{hh}", tag=f"bcs{hh}")
                            nc.sync.dma_start(bcs[:], rs2[:].broadcast_to([HD, SQQ]))
                            nc.gpsimd.tensor_mul(
                                oTq[pr][qu][row : row + HD, :], pvs[0:HD, :], bcs[:]
                            )

                # ---------------- tail remainder -------------------------
                # out-projection units not absorbed into pair 3; PSUM from
                # freed attention tags (no pool barrier), bias alternating
                # ACT/DVE (both idle now), tc ascending so the units gated
                # on the last qu's normalize come last.
                while _tail_piece(False):
                    pass

    if split_waits:
        _split_excess_waits(nc, max_waits=1)
    return nc


def _get_nc():
    if "nc" not in _CACHE:
        _CACHE["nc"] = _build()
    return _CACHE["nc"]


# --------------------------------------------------------------------------
# host entry point
# --------------------------------------------------------------------------
def _shard_inputs(x, w_qkv, b_qkv, w_out, b_out):
    import ml_dtypes

    f = np.float32
    bf = np.dtype(ml_dtypes.bfloat16)
    x = np.asarray(x, f)
    w_qkv = np.asarray(w_qkv, f)
    b_qkv = np.asarray(b_qkv, f)
    w_out = np.asarray(w_out, f)
    b_out = np.asarray(b_out, f)
    in_maps = []
    for c in range(NCORES):
        b, g = divmod(c, 2)
        cols = slice(DG * g, DG * (g + 1))
        wq_c = w_qkv[:, 0 * D :][:, cols][:, :DG] * np.float32(SCALE)
        wk_c = w_qkv[:, D : 2 * D][:, cols]
        wqk_c = np.ascontiguousarray(
            np.concatenate([wq_c, wk_c], axis=1).astype(bf)
        )
        # pass-1-critical repacks (d-major, 2KB DMA lines): pair-0 q/k cols
        # and the full wv
        wq0p_c = np.ascontiguousarray(
            np.concatenate([wq_c[:, 0:128], wk_c[:, 0:128]], axis=1)
            .astype(bf)
            .reshape(8, 128, 256)
            .transpose(1, 0, 2)
            .reshape(128, 2048)
        )
        wv_c = w_qkv[:, 2 * D :][:, cols].astype(bf)
        wvpk_c = np.ascontiguousarray(
            wv_c.reshape(8, 128, DG).transpose(1, 0, 2).reshape(128, 8 * DG)
        )
        bq_c = (b_qkv[0 * D : 1 * D][cols] * np.float32(SCALE)).reshape(4, 128).T
        bk_c = b_qkv[D : 2 * D][cols].reshape(4, 128).T
        bqk_c = np.ascontiguousarray(np.concatenate([bq_c, bk_c], axis=1), f)
        bv_c = np.ascontiguousarray(np.tile(b_qkv[2 * D :][cols], (128, 1)), f)
        wo_c = np.ascontiguousarray(w_out[DG * g : DG * (g + 1), :])
        bo_c = (
            np.ascontiguousarray(b_out.reshape(D // 128, 128).T, f)
            if g == 0
            else np.zeros((128, D // 128), f)
        )
        in_maps.append(
            {
                "xb": np.ascontiguousarray(x[b].T.astype(bf)),
                "wqk": wqk_c,
                "wq0p": wq0p_c,
                "wvpk": wvpk_c,
                "bqk": bqk_c,
                "bv": bv_c,
                "wo": wo_c,
                "bo": bo_c,
            }
        )
    return in_maps


def _patch_ldw_opt():
    """Flip walrus --enable-ldw-opt to true (dedupe repeated LDWEIGHTS for
    consecutive same-stationary matmuls). Off by default: the bf16 matmuls
    now lower to standalone InstLdweights, which walrus rejects under
    ldw-opt. Controlled by KERNEL_LDW_OPT env."""
    import os
    if os.environ.get("KERNEL_LDW_OPT", "0") != "1":
        return
    if _CACHE.get("ldw_patched"):
        return
    import concourse.bass_utils as bu

    orig = bu.run_command

    def run_command_ldw(argv, **kwargs):
        argv = [a.replace("--enable-ldw-opt=false", "--enable-ldw-opt=true")
                if isinstance(a, str) else a for a in argv]
        return orig(argv, **kwargs)

    bu.run_command = run_command_ldw
    _CACHE["ldw_patched"] = True


def kernel(x, w_qkv, b_qkv, w_out, b_out, _trace=False, _trace_kwargs=None):
    from concourse.bass_utils import run_bass_kernel_spmd

    _patch_ldw_opt()
    nc = _get_nc()
    in_maps = _shard_inputs(x, w_qkv, b_qkv, w_out, b_out)
    kw = {}
    if _trace:
        kw["trace"] = True
        kw.update(_trace_kwargs or {})
    res = run_bass_kernel_spmd(nc, in_maps, core_ids=list(range(NCORES)), **kw)
    _CACHE["last_result"] = res
    # [D, S] bf16 per core
    parts = [np.asarray(r["outp"], dtype=np.float32) for r in res.results]
    out = np.stack([(parts[2 * b] + parts[2 * b + 1]).T for b in range(B)])
    return np.ascontiguousarray(out, np.float32)



# revision 38
# speedup vs baseline: 1.0060x; 1.0060x over previous
"""Multi-head attention (B=4, S=2048, D=1024, H=16) on 8 Trainium2 NeuronCores.

Sharding: core c -> batch c//2, head-group c%2 (8 heads = 512 dims each).
Each core computes qkv projection, softmax attention and its partial
out-projection (Megatron row-split of w_out); the host sums core pairs.

All projection/attention operands are bf16 with fp32 PSUM accumulation
(x, qkv weights, qT/kT, v, exp(scores)); only the out-projection stays
float32r.  bf16 halves DMA bytes (the DMA engines are packet-rate bound:
~288ns per <=2KB line, so all big transfers use >=2KB lines) and makes
every hot-loop LDWEIGHTS a standalone fast-load that hides behind
matmuls.  x lives resident in SBUF (4MB bf16) so phase-B fillers do no
DMA.  Softmax needs no max-subtraction (scores ~ N(0,1)); denominators
come free from an augmented ones-column in V; the reciprocal runs on a
[128,4] DRAM-reshaped layout (DVE reciprocal cost scales with free
size), and the normalization multiply runs on the idle GPSIMD.

Schedule (profiled ~366us; PE-timeline-bound: ~786K PE cycles at
2.4GHz under a ~95% power-throttle duty ceiling, ACT ~78% busy):
  - pass 1 (~51us): one sweep over resident x computes v (all heads) +
    q/k (pair 0); loads spread over all three DMA-capable queues
    (sync/scalar/gpsimd run concurrently, ~400GB/s aggregate); chunk 0
    interleaves qk+v per d in DMA-arrival order, later chunks run all
    qk matmuls first so the previous chunk's psv drains overlap them.
  - phase B (~272us): per head-pair, scores (row-tiled 2-head pack) ->
    one exp per [128,1024] psum tile -> pv accumulation lagging 2
    iterations (pv LDWEIGHTS carries no pending wait); filler matmuls
    project the next pair's q/k from resident x in the PE slack of
    pairs 0-2 (PE-bound at 1066ns/iter vs 1038ns exp).  Attention
    output lands in per-(pair, 512-col q-chunk) oT tiles so the
    out-projection can consume chunks as they normalize.
  - out-projection (32 units of (dout-block j, q-chunk tc): 4 po
    matmuls + bias + [128,512] bf16 store): ~20 matmuls absorbed into
    pair 3's ACT-bound slack (no qk fillers there; 6 insert slots/qu,
    psum from the idle filler tags, DVE bias, gpsimd-queue DMA); the
    remaining ~110 run densely after the pr loop on freed attention
    psum tags, bias alternating ACT/DVE, stores alternating sync/
    scalar queues, tc ascending so the last-normalized chunk is last.
"""

import numpy as np

B, S, D, H = 4, 2048, 1024, 16
HD = D // H          # 64
HG = H // 2          # 8 heads per core
DG = HG * HD         # 512 local head-cat dims
SCALE = HD ** -0.5   # folded into wq host-side
NCORES = 8

_CACHE = {}


# --------------------------------------------------------------------------
# wait splitting: this toolchain's walrus rejects >1 sync wait per instruction
# on some paths; move excess semaphore waits onto same-engine NoOps.
# --------------------------------------------------------------------------
def _split_excess_waits(nc, max_waits=1):
    import bass_rust
    import concourse.mybir as mybir

    ctr = [0]
    for fn in nc.m.functions:
        for bb in fn.blocks:
            insts = list(bb.instructions)
            out = []
            changed = False
            for inst in insts:
                si = inst.sync_info
                waits = list(si.on_wait) if si is not None and si.on_wait else []
                sem_waits = [w for w in waits if w.sync_type == "semaphore"]
                other = [w for w in waits if w.sync_type != "semaphore"]
                budget = max_waits - len(other)
                if len(sem_waits) > budget and budget >= 1:
                    head, keep = sem_waits[:-budget], sem_waits[-budget:]
                    chunks = [
                        head[i : i + max_waits]
                        for i in range(0, len(head), max_waits)
                    ]
                    for ch in chunks:
                        nop = mybir.InstNoOp(
                            name=f"wsplit-{ctr[0]}",
                            opcode="NoOp",
                            engine=inst.engine,
                            ins=[],
                            outs=[],
                        )
                        nop.sync_info = bass_rust.SyncInfo(on_wait=ch, on_update=[])
                        ctr[0] += 1
                        out.append(nop)
                    inst.sync_info = bass_rust.SyncInfo(
                        on_wait=other + keep,
                        on_update=list(si.on_update) if si.on_update else [],
                    )
                    changed = True
                out.append(inst)
            if changed:
                bb.instructions = out


# --------------------------------------------------------------------------
# device program (identical on all 8 cores)
# --------------------------------------------------------------------------
def _build(split_waits=True):
    import concourse.bass as bass
    import concourse.tile as tile
    import concourse.mybir as mybir

    F32 = mybir.dt.float32
    F32R = mybir.dt.float32r
    BF16 = mybir.dt.bfloat16
    EXP = mybir.ActivationFunctionType.Exp
    IDENT = mybir.ActivationFunctionType.Identity
    ts = bass.ts

    nc = bass.Bass()

    xb = nc.dram_tensor("xb", [D, S], BF16, kind="ExternalInput")
    # wqk DRAM layout: cols 0:512 = wq (4 pairs x 128), 512:1024 = wk;
    # consumed only by phase-B fillers, so it loads off the critical path.
    wqk = nc.dram_tensor("wqk", [D, 2 * DG], BF16, kind="ExternalInput")
    # pass-1-critical weights, host-repacked d-major so chunk 0's blocking
    # DMA is 0.5MB+1MB of 2KB lines instead of 3MB:
    #   wq0p[p, d*256+c]  = pair-0 q (c<128) / k (c-128) col c, row d*128+p
    #   wvpk[p, d*512+c]  = wv col c, row d*128+p
    wq0p = nc.dram_tensor("wq0p", [128, (D // 128) * 256], BF16, kind="ExternalInput")
    wvpk = nc.dram_tensor("wvpk", [128, (D // 128) * DG], BF16, kind="ExternalInput")
    bqk = nc.dram_tensor("bqk", [128, 8], F32, kind="ExternalInput")
    bv = nc.dram_tensor("bv", [128, DG], F32, kind="ExternalInput")
    wo = nc.dram_tensor("wo", [DG, D], F32R, kind="ExternalInput")
    bo = nc.dram_tensor("bo", [128, D // 128], F32, kind="ExternalInput")
    outp = nc.dram_tensor("outp", [D, S], BF16, kind="ExternalOutput")

    NSQT = S // 128          # 16 sq/sk tiles of 128
    NDT = D // 128           # 8 contraction tiles
    NPAIR = HG // 2          # 4 head pairs
    VW = HD + 1              # 65: v columns + ones column per head
    SQQ = 512                # sq quarter per pv accumulation

    with tile.TileContext(nc) as tc:
        with (
            tc.tile_pool(name="bias", bufs=1) as bias_pool,
            tc.tile_pool(name="vaug", bufs=1) as v_pool,
            tc.tile_pool(name="oT", bufs=1) as oT_pool,
            tc.tile_pool(name="qkp", bufs=1) as qkp,      # rotating qT/kT slots
            tc.tile_pool(name="wqkp", bufs=1) as wqkp,
            tc.tile_pool(name="xs2", bufs=1) as xs2,      # resident bf16 x [D,S]
            tc.tile_pool(name="psqk", bufs=1, space="PSUM") as ps_qk,  # pa,pb
        ):
            bqk_t = bias_pool.tile([128, 8], F32)
            bv_t = bias_pool.tile([128, DG], F32)
            ones8_f = bias_pool.tile([128, 8], F32)
            nc.vector.memset(ones8_f[:], 1.0)
            ones8 = bias_pool.tile([128, 8], BF16)
            nc.vector.tensor_copy(ones8[:], ones8_f[:])
            actwarm = bias_pool.tile([1, 1], F32)

            # wqk_t[d]: cols 0:512 = wq (SCALE folded), 512:1024 = wk
            # (whole-tile loads: 2KB DMA lines; splitting by pair halves the
            # critical bytes but quadruples packets -> slower, measured)
            wqk_t = [
                wqkp.tile([128, 2 * DG], BF16, name=f"wqk{d}", tag=f"wqk{d}")
                for d in range(NDT)
            ]

            v_t = [v_pool.tile([128, HG * VW], BF16, name=f"v{s}", tag=f"v{s}") for s in range(NSQT)]
            # attention output, one tile per (pair, 512-col q-chunk) so the
            # out-projection can start on a chunk as soon as its normalize
            # lands (tile-granular deps)
            oTq = [
                [
                    oT_pool.tile([128, SQQ], F32R, name=f"oT{p}_{qc}", tag=f"oT{p}_{qc}")
                    for qc in range(S // SQQ)
                ]
                for p in range(NPAIR)
            ]
            qT_t = [qkp.tile([128, S], BF16, name=f"qT{i}", tag=f"qT{i}") for i in range(2)]
            kT_t = [qkp.tile([128, S], BF16, name=f"kT{i}", tag=f"kT{i}") for i in range(2)]

            # ------- pass 1: single sweep over x -> v(all) + qk(pair0) -------
            # x arrives as [128,1024] bf16 tiles (2KB DMA lines), each
            # resident across the two 512-col chunks that consume it; per
            # chunk+d we issue 6 back-to-back matmuls.  PSUM: gq/gk double-
            # buffered across chunks (pa,pb | pg,ph) + 4 v banks = 8.
            with (
                tc.tile_pool(name="wvp", bufs=1) as wvp,
                tc.tile_pool(name="psv", bufs=1, space="PSUM") as ps_v,
                tc.tile_pool(name="psg", bufs=1, space="PSUM") as ps_g,
            ):
                # packed pass-1 weights, 2 tiles each (d 0-3 / d 4-7) so the
                # second half's arrival doesn't gate the first half's use
                wq0_t = [
                    wvp.tile([128, 4 * 256], BF16, name=f"wq0_{j}", tag=f"wq0_{j}")
                    for j in range(2)
                ]
                # wv in 4 d-pair tiles: fine enough that the first psv
                # matmul doesn't wait a megabyte, coarse enough for 2KB lines
                wvk_t = [
                    wvp.tile([128, 2 * DG], BF16, name=f"wvk{j}", tag=f"wvk{j}")
                    for j in range(4)
                ]
                # x stays resident in SBUF for the whole kernel (the phase-B
                # fillers then need no DMA at all); weight and x loads are
                # emitted d-interleaved in demand order across both hw queues
                xr_t = [
                    [
                        xs2.tile([128, 1024], BF16, name=f"x2_{cc}_{d}", tag=f"x{cc}_{d}")
                        for d in range(NDT)
                    ]
                    for cc in range(S // 1024)
                ]
                # three concurrent DMA queues (sync/scalar/gpsimd-SWDGE
                # aggregate ~360GB/s); cc0-critical tensors first, cc1's x
                # trails so it never competes with the cc0 window
                # chunk-0-critical stream: packed qk-pair0 + packed wv +
                # x chunk 0, ~3.5MB of 2KB lines vs ~10us of chunk-0 compute
                nc.sync.dma_start(wq0_t[0][:], wq0p[:, 0:1024])
                nc.gpsimd.dma_start(wvk_t[0][:], wvpk[:, 0 : 2 * DG])
                for d in range(NDT):
                    eng = nc.sync if d % 2 == 0 else nc.scalar
                    eng.dma_start(xr_t[0][d][:], xb[ts(d, 128), ts(0, 1024)])
                    if d == 1:
                        nc.scalar.dma_start(wq0_t[1][:], wq0p[:, 1024:2048])
                        nc.gpsimd.dma_start(wvk_t[1][:], wvpk[:, 2 * DG : 4 * DG])
                    if d == 3:
                        nc.gpsimd.dma_start(wvk_t[2][:], wvpk[:, 4 * DG : 6 * DG])
                        nc.gpsimd.dma_start(wvk_t[3][:], wvpk[:, 6 * DG : 8 * DG])
                for d in range(NDT):
                    eng = nc.sync if d % 2 == 0 else nc.scalar
                    eng.dma_start(xr_t[1][d][:], xb[ts(d, 128), ts(1, 1024)])
                # non-critical loads behind the pass-1 stream: biases (first
                # needed at the ci=0 drain), the one-time exp table preload
                # (~2.7us, must land before phase B), and the filler-only
                # full wqk tiles (first needed at phase B, ~60us in)
                nc.gpsimd.dma_start(bqk_t[:], bqk[:, :])
                nc.gpsimd.dma_start(bv_t[:], bv[:, :])
                nc.scalar.activation(actwarm[:], ones8_f[0:1, 0:1], EXP)
                for d in range(NDT):
                    eng = nc.sync if d % 2 == 0 else nc.scalar
                    eng.dma_start(wqk_t[d][:], wqk[ts(d, 128), :])

                for cc in range(S // 1024):
                    x2_t = xr_t[cc]
                    for sub in range(2):
                        ci = 2 * cc + sub
                        if ci % 2 == 0:
                            gq = ps_qk.tile([128, 512], F32, name="gq", tag="pa")
                            gk = ps_qk.tile([128, 512], F32, name="gk", tag="pb")
                        else:
                            gq = ps_g.tile([128, 512], F32, name="gq", tag="pg")
                            gk = ps_g.tile([128, 512], F32, name="gk", tag="ph")
                        psv = [
                            ps_v.tile([128, DG], F32, name="psv", tag=t)
                            for t in ("pc", "pd", "pe", "pf")
                        ]
                        def _wq0(d):
                            return wq0_t[d // 4][:, (d % 4) * 256 : (d % 4) * 256 + 128]

                        def _wk0(d):
                            return wq0_t[d // 4][:, (d % 4) * 256 + 128 : (d % 4) * 256 + 256]

                        def _wv(d):
                            return wvk_t[d // 2][:, (d % 2) * DG : (d % 2 + 1) * DG]

                        if ci == 0:
                            # chunk 0 is DMA-paced: consume tensors in
                            # arrival order, qk+v interleaved per d
                            for d in range(NDT):
                                xch = x2_t[d][:, ts(sub, 512)]
                                nc.tensor.matmul(
                                    gq[:], _wq0(d), xch,
                                    start=(d == 0), stop=(d == NDT - 1),
                                )
                                nc.tensor.matmul(
                                    gk[:], _wk0(d), xch,
                                    start=(d == 0), stop=(d == NDT - 1),
                                )
                                for si in range(4):
                                    nc.tensor.matmul(
                                        psv[si][:], xch[:, ts(si, 128)], _wv(d),
                                        start=(d == 0), stop=(d == NDT - 1),
                                    )
                        else:
                            # later chunks: qk first so the previous chunk's
                            # psv drains (4 x 683ns on DVE) overlap ~3.4us of
                            # qk matmuls instead of stalling the psv writes
                            for d in range(NDT):
                                xch = x2_t[d][:, ts(sub, 512)]
                                nc.tensor.matmul(
                                    gq[:], _wq0(d), xch,
                                    start=(d == 0), stop=(d == NDT - 1),
                                )
                                nc.tensor.matmul(
                                    gk[:], _wk0(d), xch,
                                    start=(d == 0), stop=(d == NDT - 1),
                                )
                            for d in range(NDT):
                                xch = x2_t[d][:, ts(sub, 512)]
                                for si in range(4):
                                    nc.tensor.matmul(
                                        psv[si][:], xch[:, ts(si, 128)], _wv(d),
                                        start=(d == 0), stop=(d == NDT - 1),
                                    )
                        # drains: v-aug first (next chunk's psv matmuls wait
                        # on these); qk bias adds last (banks double-buffered)
                        for si in range(4):
                            s = 4 * ci + si
                            vap = v_t[s][:].rearrange("p (h e) -> p h e", e=VW)
                            nc.vector.tensor_add(
                                vap[:, :, 0:HD],
                                psv[si][:].rearrange("p (h e) -> p h e", e=HD),
                                bv_t[:].rearrange("p (h e) -> p h e", e=HD),
                            )
                            nc.gpsimd.tensor_copy(
                                vap[:, :, HD : HD + 1], ones8[:, :, None]
                            )
                        nc.vector.tensor_scalar_add(
                            qT_t[0][:, ts(ci, 512)], gq[:], bqk_t[:, 0:1]
                        )
                        nc.vector.tensor_scalar_add(
                            kT_t[0][:, ts(ci, 512)], gk[:], bqk_t[:, 4:5]
                        )

            # ---------------- phase B: attention + fillers + tail ---------
            # Two heads share one [128,1024] scores psum tile; one exp covers
            # both heads.  pv matmuls are software-pipelined one step behind.
            # The next pair's q/k projection fills the PE slack under the
            # ACT-bound exp stream; x tiles are re-read as [128,1024] bf16
            # spanning two qu's.  The out-projection tail lives inside this
            # pool scope (reusing attention PSUM tags) so no pool-exit
            # barrier precedes it.
            with (
                tc.tile_pool(name="pt", bufs=3) as ptp,
                tc.tile_pool(name="scp", bufs=2, space="PSUM") as scp,
                tc.tile_pool(name="pvp", bufs=1, space="PSUM") as pvp,
                tc.tile_pool(name="nrm", bufs=1) as nrm,
                tc.tile_pool(name="pvs", bufs=1) as pvsp,
                tc.tile_pool(name="rs", bufs=4, space="DRAM") as rsp,
                tc.tile_pool(name="w3", bufs=1) as w3,
                tc.tile_pool(name="outb", bufs=6) as outb,
            ):
                wo_t = [
                    w3.tile([128, D], F32R, name=f"wo{pp}", tag=f"wo{pp}")
                    for pp in range(NPAIR)
                ]
                bo_t = outb.tile([128, NDT], F32, name="bo_t", tag="bo_t")

                # ---- out-projection emitter ---------------------------------
                # 32 units (j, tc): po[128,512] accumulated over the 4 pairs,
                # bias-add, [128,512] bf16 store.  Units whose oT chunks are
                # ready are absorbed one matmul per s-iteration into pair 3's
                # ACT-bound slack (pair 3 has no qk fillers); the rest run
                # densely after the pr loop with ACT free for bias adds.
                t_units = [(j, tc) for tc in range(S // SQQ) for j in range(NDT)]
                t_st = {"ui": 0, "pp": 0, "nd": 0, "po": None}
                slots_p3 = [(ps_qk, "pa"), (ps_qk, "pb")]
                slots_rem = [
                    (scp, "sc"), (ps_qk, "pa"), (ps_qk, "pb"),
                    (pvp, "pv0"), (pvp, "pv1"),
                ]

                def _tail_piece(in_p3, qu=None, s=None):
                    if t_st["ui"] >= len(t_units):
                        return False
                    j, tc = t_units[t_st["ui"]]
                    pp = t_st["pp"]
                    if in_p3 and pp == 0:
                        # unit-start gate: pair-3's normalize for chunk tc
                        # lands ~5-6 iterations into qu=tc+1; don't let an
                        # in-order PE wait stall the score stream
                        if not (tc <= qu - 2 or (tc == qu - 1 and s >= 4)):
                            return False
                    if pp == 0:
                        pool, tag = (slots_p3 if in_p3 else slots_rem)[
                            t_st["nd"] % (2 if in_p3 else 5)
                        ]
                        t_st["po"] = pool.tile(
                            [128, SQQ], F32, name=f"po{j}_{tc}", tag=tag
                        )
                    nc.tensor.matmul(
                        t_st["po"][:],
                        wo_t[pp][:, ts(j, 128)],
                        oTq[pp][tc][:],
                        start=(pp == 0), stop=(pp == NPAIR - 1),
                    )
                    if pp == NPAIR - 1:
                        ob = outb.tile(
                            [128, SQQ], BF16, name=f"ob{j}_{tc}", tag="ob"
                        )
                        if in_p3 or t_st["nd"] % 2:
                            nc.vector.tensor_scalar_add(
                                ob[:], t_st["po"][:], bo_t[:, j : j + 1]
                            )
                        else:
                            nc.scalar.activation(
                                ob[:], t_st["po"][:], IDENT,
                                bias=bo_t[:, j : j + 1],
                            )
                        # absorbed units must not touch the ACT engine (exp
                        # stream) nor the sync queue (normalize chains live
                        # there); the remainder alternates sync/scalar so
                        # output transfers drain twice as fast
                        dq = (
                            nc.gpsimd
                            if in_p3
                            else (nc.sync if t_st["nd"] % 2 else nc.scalar)
                        )
                        dq.dma_start(
                            outp[ts(j, 128), tc * SQQ : (tc + 1) * SQQ], ob[:]
                        )
                        t_st["nd"] += 1
                        t_st["ui"] += 1
                        t_st["pp"] = 0
                    else:
                        t_st["pp"] = pp + 1
                    return True

                for pr in range(NPAIR):
                    h0, h1 = 2 * pr, 2 * pr + 1
                    qTc, kTc = qT_t[pr % 2], kT_t[pr % 2]
                    nxt = pr + 1 if pr + 1 < NPAIR else None
                    if pr == 2:
                        # wo on sync: its phase-B program is semaphore-paced,
                        # so the transfer lands here and not during pass 1
                        # (gpsimd would issue it immediately)
                        for pp in range(NPAIR):
                            nc.sync.dma_start(wo_t[pp][:], wo[ts(pp, 128), :])
                        nc.gpsimd.dma_start(bo_t[:], bo[:, :])
                    for qu in range(S // SQQ):
                        qs = slice(qu * SQQ, (qu + 1) * SQQ)
                        pv0 = pvp.tile([VW, SQQ], F32, name="pv0", tag="pv0")
                        pv1 = pvp.tile([VW, SQQ], F32, name="pv1", tag="pv1")
                        if nxt is not None:
                            gq = ps_qk.tile([128, 512], F32, name="gq", tag="pa")
                            gk = ps_qk.tile([128, 512], F32, name="gk", tag="pb")
                        # pv matmuls run 2 iterations behind their exp so the
                        # pv LDWEIGHTS carries no pending wait and hides
                        # behind in-flight matmuls
                        pt_q = []
                        for s in range(NSQT):
                            if nxt is not None:
                                # filler: qk projection for the next pair,
                                # 1 matmul/iter from the resident x tiles,
                                # finishing (with bias adds) at s==14
                                if s < 12:
                                    d = s // 2
                                    xch = xr_t[qu // 2][d][:, ts(qu % 2, 512)]
                                    if s % 2 == 0:
                                        nc.tensor.matmul(
                                            gq[:], wqk_t[d][:, ts(nxt, 128)], xch,
                                            start=(d == 0), stop=False,
                                        )
                                    else:
                                        nc.tensor.matmul(
                                            gk[:],
                                            wqk_t[d][:, DG + nxt * 128 : DG + (nxt + 1) * 128],
                                            xch,
                                            start=(d == 0), stop=False,
                                        )
                                elif s in (12, 13):
                                    d = s - 6
                                    xch = xr_t[qu // 2][d][:, ts(qu % 2, 512)]
                                    nc.tensor.matmul(
                                        gq[:], wqk_t[d][:, ts(nxt, 128)], xch,
                                        start=False, stop=(d == NDT - 1),
                                    )
                                    nc.tensor.matmul(
                                        gk[:],
                                        wqk_t[d][:, DG + nxt * 128 : DG + (nxt + 1) * 128],
                                        xch,
                                        start=False, stop=(d == NDT - 1),
                                    )
                                    if s == 13:
                                        nc.vector.tensor_scalar_add(
                                            qT_t[nxt % 2][:, ts(qu, 512)], gq[:],
                                            bqk_t[:, nxt : nxt + 1],
                                        )
                                        nc.vector.tensor_scalar_add(
                                            kT_t[nxt % 2][:, ts(qu, 512)], gk[:],
                                            bqk_t[:, 4 + nxt : 5 + nxt],
                                        )
                            elif s in (3, 5, 7, 9, 11, 13):
                                # pair 3 is ACT-bound (~12us of PE slack
                                # total): absorb out-projection matmuls for
                                # already-normalized q-chunks, few enough to
                                # keep the exp stream the pacer
                                _tail_piece(True, qu=qu, s=s)
                            sc = scp.tile([128, 2 * SQQ], F32, name="sc", tag="sc")
                            nc.tensor.matmul(
                                sc[:, 0:SQQ],
                                kTc[0:HD, ts(s, 128)],
                                qTc[0:HD, qs],
                                start=True, stop=True,
                            )
                            nc.tensor.matmul(
                                sc[:, SQQ : 2 * SQQ],
                                kTc[HD:128, ts(s, 128)],
                                qTc[HD:128, qs],
                                start=True, stop=True,
                            )
                            pt = ptp.tile([128, 2 * SQQ], BF16, name="pt", tag="pt")
                            nc.scalar.activation(pt[:], sc[:], EXP)
                            if len(pt_q) == 2:
                                pp_, ps_ = pt_q.pop(0)
                                nc.tensor.matmul(
                                    pv0[:], v_t[ps_][:, h0 * VW : (h0 + 1) * VW],
                                    pp_[:, 0:SQQ],
                                    start=(ps_ == 0), stop=False,
                                )
                                nc.tensor.matmul(
                                    pv1[:], v_t[ps_][:, h1 * VW : (h1 + 1) * VW],
                                    pp_[:, SQQ : 2 * SQQ],
                                    start=(ps_ == 0), stop=False,
                                )
                            pt_q.append((pt, s))
                        for pp_, ps_ in pt_q:
                            nc.tensor.matmul(
                                pv0[:], v_t[ps_][:, h0 * VW : (h0 + 1) * VW],
                                pp_[:, 0:SQQ],
                                start=False, stop=(ps_ == NSQT - 1),
                            )
                            nc.tensor.matmul(
                                pv1[:], v_t[ps_][:, h1 * VW : (h1 + 1) * VW],
                                pp_[:, SQQ : 2 * SQQ],
                                start=False, stop=(ps_ == NSQT - 1),
                            )
                        # free pv banks via psum->sbuf copy, then normalize.
                        # The denominator row [1,512] is reshaped to [128,4]
                        # through DRAM so the DVE reciprocal costs ~0.17us
                        # instead of 3.3us (DVE reciprocal is ~6.5 cyc/elem
                        # and free-size-serial); the multiply runs on GPSIMD.
                        for hh, pvx, row in ((0, pv0, 0), (1, pv1, HD)):
                            pvs = pvsp.tile([VW, SQQ], F32, name=f"pvs{hh}", tag=f"pvs{hh}")
                            nc.vector.tensor_copy(pvs[:], pvx[:])
                            rsd = rsp.tile([1, SQQ], F32, name=f"rsd{hh}", tag=f"rsd{hh}")
                            nc.sync.dma_start(rsd[:], pvs[HD : HD + 1, :])
                            dent = nrm.tile([128, 4], F32, name=f"dent{hh}", tag=f"dent{hh}")
                            nc.sync.dma_start(dent[:], rsd[:])
                            rcd = nrm.tile([128, 4], F32, name=f"rcd{hh}", tag=f"rcd{hh}")
                            nc.vector.reciprocal(rcd[:], dent[:])
                            rs2 = rsp.tile([1, SQQ], F32, name=f"rs2{hh}", tag=f"rs2{hh}")
                            nc.sync.dma_start(rs2[:], rcd[:])
                            bcs = nrm.tile([HD, SQQ], F32, name=f"bcs{hh}", tag=f"bcs{hh}")
                            nc.sync.dma_start(bcs[:], rs2[:].broadcast_to([HD, SQQ]))
                            nc.gpsimd.tensor_mul(
                                oTq[pr][qu][row : row + HD, :], pvs[0:HD, :], bcs[:]
                            )

                # ---------------- tail remainder -------------------------
                # out-projection units not absorbed into pair 3; PSUM from
                # freed attention tags (no pool barrier), bias alternating
                # ACT/DVE (both idle now), tc ascending so the units gated
                # on the last qu's normalize come last.
                while _tail_piece(False):
                    pass

    if split_waits:
        _split_excess_waits(nc, max_waits=1)
    return nc


def _get_nc():
    if "nc" not in _CACHE:
        _CACHE["nc"] = _build()
    return _CACHE["nc"]


# --------------------------------------------------------------------------
# host entry point
# --------------------------------------------------------------------------
def _shard_inputs(x, w_qkv, b_qkv, w_out, b_out):
    import ml_dtypes

    f = np.float32
    bf = np.dtype(ml_dtypes.bfloat16)
    x = np.asarray(x, f)
    w_qkv = np.asarray(w_qkv, f)
    b_qkv = np.asarray(b_qkv, f)
    w_out = np.asarray(w_out, f)
    b_out = np.asarray(b_out, f)
    in_maps = []
    for c in range(NCORES):
        b, g = divmod(c, 2)
        cols = slice(DG * g, DG * (g + 1))
        wq_c = w_qkv[:, 0 * D :][:, cols][:, :DG] * np.float32(SCALE)
        wk_c = w_qkv[:, D : 2 * D][:, cols]
        wqk_c = np.ascontiguousarray(
            np.concatenate([wq_c, wk_c], axis=1).astype(bf)
        )
        # pass-1-critical repacks (d-major, 2KB DMA lines): pair-0 q/k cols
        # and the full wv
        wq0p_c = np.ascontiguousarray(
            np.concatenate([wq_c[:, 0:128], wk_c[:, 0:128]], axis=1)
            .astype(bf)
            .reshape(8, 128, 256)
            .transpose(1, 0, 2)
            .reshape(128, 2048)
        )
        wv_c = w_qkv[:, 2 * D :][:, cols].astype(bf)
        wvpk_c = np.ascontiguousarray(
            wv_c.reshape(8, 128, DG).transpose(1, 0, 2).reshape(128, 8 * DG)
        )
        bq_c = (b_qkv[0 * D : 1 * D][cols] * np.float32(SCALE)).reshape(4, 128).T
        bk_c = b_qkv[D : 2 * D][cols].reshape(4, 128).T
        bqk_c = np.ascontiguousarray(np.concatenate([bq_c, bk_c], axis=1), f)
        bv_c = np.ascontiguousarray(np.tile(b_qkv[2 * D :][cols], (128, 1)), f)
        wo_c = np.ascontiguousarray(w_out[DG * g : DG * (g + 1), :])
        bo_c = (
            np.ascontiguousarray(b_out.reshape(D // 128, 128).T, f)
            if g == 0
            else np.zeros((128, D // 128), f)
        )
        in_maps.append(
            {
                "xb": np.ascontiguousarray(x[b].T.astype(bf)),
                "wqk": wqk_c,
                "wq0p": wq0p_c,
                "wvpk": wvpk_c,
                "bqk": bqk_c,
                "bv": bv_c,
                "wo": wo_c,
                "bo": bo_c,
            }
        )
    return in_maps


def _patch_ldw_opt():
    """Flip walrus --enable-ldw-opt to true (dedupe repeated LDWEIGHTS for
    consecutive same-stationary matmuls). Off by default: the bf16 matmuls
    now lower to standalone InstLdweights, which walrus rejects under
    ldw-opt. Controlled by KERNEL_LDW_OPT env."""
    import os
    if os.environ.get("KERNEL_LDW_OPT", "0") != "1":
        return
    if _CACHE.get("ldw_patched"):
        return
    import concourse.bass_utils as bu

    orig = bu.run_command

    def run_command_ldw(argv, **kwargs):
        argv = [a.replace("--enable-ldw-opt=false", "--enable-ldw-opt=true")
                if isinstance(a, str) else a for a in argv]
        return orig(argv, **kwargs)

    bu.run_command = run_command_ldw
    _CACHE["ldw_patched"] = True


def kernel(x, w_qkv, b_qkv, w_out, b_out, _trace=False, _trace_kwargs=None):
    from concourse.bass_utils import run_bass_kernel_spmd

    _patch_ldw_opt()
    nc = _get_nc()
    in_maps = _shard_inputs(x, w_qkv, b_qkv, w_out, b_out)
    kw = {}
    if _trace:
        kw["trace"] = True
        kw.update(_trace_kwargs or {})
    res = run_bass_kernel_spmd(nc, in_maps, core_ids=list(range(NCORES)), **kw)
    _CACHE["last_result"] = res
    # [D, S] bf16 per core
    parts = [np.asarray(r["outp"], dtype=np.float32) for r in res.results]
    out = np.stack([(parts[2 * b] + parts[2 * b + 1]).T for b in range(B)])
    return np.ascontiguousarray(out, np.float32)

